# revision 14
# baseline (speedup 1.0000x reference)
"""ChessNNUE Trainium2 kernel.

Fast path (sparse embedding-lookup; used when the feature matrices are
binary 0/1, which they are for HalfKP-style NNUE inputs):
  - The dense FT matmul (85.9 GFLOP/core -> 1.09 ms PE-bound) is replaced
    by hardware gather: each batch row has ~30 active features, so
      acc[b, :] = ft_b + sum_{k active} ft_w.T[k, :]
  - Host extracts active indices, bin-packs the 4096 rows into 32
    (core, batch-tile) bins of 128 rows to equalize stream lengths, and
    builds int16 gather-index streams (split into a <32767 "low" table
    and a "high" table because dma_gather indices are int16).
  - On device, nc.gpsimd.dma_gather pulls the active bf16 table rows
    into SBUF ([128 slots, G, 1024]); a 0/1 segment matrix S (built on
    DVE via iota==segid compare) reduces slots into per-row accumulators
    on the tensor engine: PSUM[b, hid] = S.T @ gathered. ft_b is folded
    in via an identity-matmul K-tile over a broadcast bias tile.
  - stm blend runs with batch on partitions (stm is a per-partition
    scalar), then h1 is PE-transposed back to [hid, batch] for the
    small MLP head (same code as the dense path).
  - HBM traffic/core drops from 335 MB to ~70 MB and PE work drops 8x.

Fallback path: the original dense data-parallel kernel (features bf16
matmul over all 40960 features), used if inputs are not binary-sparse.
"""

import math
import numpy as np
from contextlib import ExitStack

import concourse.bass as bass
import concourse.tile as tile
from concourse import bacc, mybir
from concourse.bass_utils import run_bass_kernel_spmd

B, FEAT, HID = 4096, 40960, 1024
L1, L2 = 64, 32
NCORES = 8
BC = B // NCORES          # 512 batch rows per core
BT = BC // 128            # 4 batch tiles of 128 rows per core
NHC = HID // 128          # 8 hid chunks of 128

SPLIT = 32767             # features < SPLIT go to the low table
NTA = 32768               # low table rows (ft rows 0..32766 + zero row)
NTB = FEAT - SPLIT + 1    # high table rows (ft rows 32767.. + zero row)
ZLO = NTA - 1             # zero-row index in low table
ZHI = NTB - 1             # zero-row index in high table
GCAP = 48                 # max total K-tiles per stream (SBUF budget)

F32 = mybir.dt.float32
BF16 = mybir.dt.bfloat16
I16 = mybir.dt.int16
BF16_NP = mybir.dt.np(BF16)

_CACHE = {}


# --------------------------------------------------------------------------
# Sparse (embedding lookup) path
# --------------------------------------------------------------------------

def _build_sparse(ga, gb, reps=1, mode="full", nq=4):
    """Per-core Bass program for the gather path.

    ga/gb: K-tiles (128 gather slots each) per (batch-tile, perspective)
    stream for the low/high table halves.
    mode: "full" | "dmaonly" (gathers only) | "nodma" (compute only) —
    ablation builds for timing breakdown.
    """
    Alu = mybir.AluOpType
    Act = mybir.ActivationFunctionType
    do_dma = mode in ("full", "dmaonly")
    do_pe = mode in ("full", "nodma")

    nc = bacc.Bacc("TRN2", target_bir_lowering=False, debug=False,
                   num_devices=NCORES, num_swdge_queues=nq)
    qc = [0]

    tbl = nc.dram_tensor("tbl", (NTA + NTB, HID), BF16, kind="ExternalInput")
    idxlo = nc.dram_tensor("idxlo", (128, BT * 2 * ga * 8), I16,
                           kind="ExternalInput")
    seglo = nc.dram_tensor("seglo", (128, BT * 2 * ga), F32,
                           kind="ExternalInput")
    if gb:
        idxhi = nc.dram_tensor("idxhi", (128, BT * 2 * gb * 8), I16,
                               kind="ExternalInput")
        seghi = nc.dram_tensor("seghi", (128, BT * 2 * gb), F32,
                               kind="ExternalInput")
    stm4 = nc.dram_tensor("stm4", (128, BT), F32, kind="ExternalInput")
    iota = nc.dram_tensor("iota", (128, 128), F32, kind="ExternalInput")
    ident = nc.dram_tensor("ident", (128, 128), BF16, kind="ExternalInput")
    biasb = nc.dram_tensor("biasb", (128, HID), BF16, kind="ExternalInput")
    l1w = nc.dram_tensor("l1w", (2 * HID, L1), BF16, kind="ExternalInput")
    l1b = nc.dram_tensor("l1b", (L1,), F32, kind="ExternalInput")
    l2w = nc.dram_tensor("l2w", (L1, L2), BF16, kind="ExternalInput")
    l2b = nc.dram_tensor("l2b", (L2,), F32, kind="ExternalInput")
    l3w = nc.dram_tensor("l3w", (L2, 1), BF16, kind="ExternalInput")
    l3b = nc.dram_tensor("l3b", (1,), F32, kind="ExternalInput")
    out = nc.dram_tensor("out", (2, BC), F32, kind="ExternalOutput")

    tblA = tbl.ap()[0:NTA, :]
    tblB = tbl.ap()[NTA:NTA + NTB, :]

    with ExitStack() as ctx:
        tc = ctx.enter_context(tile.TileContext(nc))
        const = ctx.enter_context(tc.tile_pool(name="const", bufs=1))
        glopool = ctx.enter_context(tc.tile_pool(name="glopool", bufs=2))
        ghipool = ctx.enter_context(tc.tile_pool(name="ghipool", bufs=2))
        spool = ctx.enter_context(tc.tile_pool(name="spool", bufs=2))
        tmppool = ctx.enter_context(tc.tile_pool(name="tmppool", bufs=1))
        h1pool = ctx.enter_context(tc.tile_pool(name="h1pool", bufs=2))
        h1Tpool = ctx.enter_context(tc.tile_pool(name="h1Tpool", bufs=1))
        psum = ctx.enter_context(
            tc.tile_pool(name="psum", bufs=6, space="PSUM"))

        # ---------- constants ----------
        iota_sb = const.tile([128, 128], F32)
        nc.sync.dma_start(iota_sb[:], iota.ap())
        ident_sb = const.tile([128, 128], BF16)
        nc.sync.dma_start(ident_sb[:], ident.ap())
        biasb_sb = const.tile([128, HID], BF16)
        nc.sync.dma_start(biasb_sb[:], biasb.ap())
        stm_sb = const.tile([128, BT], F32)
        nc.sync.dma_start(stm_sb[:], stm4.ap())
        idxlo_sb = const.tile([128, BT * 2 * ga * 8], I16)
        nc.sync.dma_start(idxlo_sb[:], idxlo.ap())
        seglo_sb = const.tile([128, BT * 2 * ga], F32)
        nc.sync.dma_start(seglo_sb[:], seglo.ap())
        if gb:
            idxhi_sb = const.tile([128, BT * 2 * gb * 8], I16)
            nc.sync.dma_start(idxhi_sb[:], idxhi.ap())
            seghi_sb = const.tile([128, BT * 2 * gb], F32)
            nc.sync.dma_start(seghi_sb[:], seghi.ap())

        l1b_sb = const.tile([L1, 1], F32)
        nc.sync.dma_start(l1b_sb[:], l1b.ap())
        l2b_sb = const.tile([L2, 1], F32)
        nc.sync.dma_start(l2b_sb[:], l2b.ap())
        l3b_sb = const.tile([1, 1], F32)
        nc.sync.dma_start(l3b_sb[:], l3b.ap())
        l1w_sb = const.tile([128, (2 * HID) // 128, L1], BF16)
        nc.sync.dma_start(l1w_sb[:],
                          l1w.ap().rearrange("(t p) m -> p t m", p=128))
        l2w_sb = const.tile([L1, L2], BF16)
        nc.sync.dma_start(l2w_sb[:], l2w.ap())
        l3w_sb = const.tile([L2, 1], BF16)
        nc.sync.dma_start(l3w_sb[:], l3w.ap())

        h1Ts = [h1Tpool.tile([128, BC], BF16, tag=f"h1T{t}", name=f"h1T{t}")
                for t in range(16)]

        def emit_body():
            for bt in range(BT):
                psb = {}
                for p in range(2):
                    sidx = bt * 2 + p
                    # HW SWDGE descriptor scratch holds 1024 descriptors ->
                    # at most 8 K-tiles (1024 idxs) per dma_gather call.
                    glo_t = glopool.tile([128, ga, HID], BF16, tag="glo",
                                         name="glo")
                    ghi_t = (ghipool.tile([128, gb, HID], BF16, tag="ghi",
                                          name="ghi") if gb else None)
                    if do_dma:
                        # DMASW sem lanes rotate per SWDGE instruction in
                        # emission order (8 lanes); queue = counter % nq
                        # (nq divides 8) keeps each lane on one queue.
                        for g0 in range(0, ga, 8):
                            g1 = min(g0 + 8, ga)
                            nc.gpsimd.dma_gather(
                                glo_t[:, g0:g1, :], tblA,
                                idxlo_sb[:, sidx * ga * 8 + g0 * 8:
                                         sidx * ga * 8 + g1 * 8],
                                (g1 - g0) * 128, (g1 - g0) * 128, HID,
                                queue_num=qc[0] % nq)
                            qc[0] += 1
                        for g0 in range(0, gb, 8):
                            g1 = min(g0 + 8, gb)
                            nc.gpsimd.dma_gather(
                                ghi_t[:, g0:g1, :], tblB,
                                idxhi_sb[:, sidx * gb * 8 + g0 * 8:
                                         sidx * gb * 8 + g1 * 8],
                                (g1 - g0) * 128, (g1 - g0) * 128, HID,
                                queue_num=qc[0] % nq)
                            qc[0] += 1

                    if not do_pe:
                        continue
                    # build all segment matrices of the stream in one DVE op
                    # each: keeps the PE burst free of cross-engine waits
                    Slo = spool.tile([128, ga, 128], BF16, tag="Slo")
                    nc.vector.tensor_tensor(
                        Slo[:],
                        iota_sb[:].rearrange("p b -> p () b")
                        .broadcast_to([128, ga, 128]),
                        seglo_sb[:, sidx * ga:(sidx + 1) * ga]
                        .rearrange("p g -> p g ()")
                        .broadcast_to([128, ga, 128]),
                        Alu.is_equal)
                    if gb:
                        Shi = spool.tile([128, gb, 128], BF16, tag="Shi")
                        nc.vector.tensor_tensor(
                            Shi[:],
                            iota_sb[:].rearrange("p b -> p () b")
                            .broadcast_to([128, gb, 128]),
                            seghi_sb[:, sidx * gb:(sidx + 1) * gb]
                            .rearrange("p g -> p g ()")
                            .broadcast_to([128, gb, 128]),
                            Alu.is_equal)
                    ps0 = psum.tile([128, 512], F32, tag="ps")
                    ps1 = psum.tile([128, 512], F32, tag="ps")
                    # ft_b K-tile: out[b, h] += sum_p I[p, b] * biasb[p, h]
                    nc.tensor.matmul(ps0[:], ident_sb[:], biasb_sb[:, 0:512],
                                     start=True, stop=False)
                    nc.tensor.matmul(ps1[:], ident_sb[:], biasb_sb[:, 512:HID],
                                     start=True, stop=False)
                    for g in range(ga):
                        last = (g == ga - 1) and gb == 0
                        nc.tensor.matmul(ps0[:], Slo[:, g, :], glo_t[:, g, 0:512],
                                         start=False, stop=last)
                        nc.tensor.matmul(ps1[:], Slo[:, g, :], glo_t[:, g, 512:HID],
                                         start=False, stop=last)
                    for g in range(gb):
                        last = g == gb - 1
                        nc.tensor.matmul(ps0[:], Shi[:, g, :], ghi_t[:, g, 0:512],
                                         start=False, stop=last)
                        nc.tensor.matmul(ps1[:], Shi[:, g, :], ghi_t[:, g, 512:HID],
                                         start=False, stop=last)
                    psb[p] = (ps0, ps1)

                # stm blend + clip (batch rows on partitions)
                for h in range(2 if do_pe else 0):
                    w_, b_ = psb[0][h], psb[1][h]
                    # walrus allows only one PSUM input per DVE op
                    wsb = tmppool.tile([128, 512], F32, tag="wsb")
                    nc.vector.tensor_copy(wsb[:], w_[:])
                    w_ = wsb
                    d = tmppool.tile([128, 512], F32, tag="d")
                    nc.vector.tensor_sub(d[:], w_[:], b_[:])
                    m = tmppool.tile([128, 512], F32, tag="m")
                    nc.vector.tensor_scalar(m[:], d[:],
                                            stm_sb[:, bt:bt + 1], None,
                                            Alu.mult)
                    topf = tmppool.tile([128, 512], F32, tag="topf")
                    nc.vector.tensor_add(topf[:], b_[:], m[:])
                    botf = tmppool.tile([128, 512], F32, tag="botf")
                    nc.vector.tensor_sub(botf[:], w_[:], m[:])
                    h1t = h1pool.tile([128, 512], BF16, tag=f"h1t{h}")
                    nc.gpsimd.tensor_scalar(h1t[:], topf[:], 0.0, 1.0,
                                            Alu.max, Alu.min)
                    h1b = h1pool.tile([128, 512], BF16, tag=f"h1b{h}")
                    nc.gpsimd.tensor_scalar(h1b[:], botf[:], 0.0, 1.0,
                                            Alu.max, Alu.min)
                    # transpose back to [hid, batch] for the head
                    for q in range(4):
                        pst = psum.tile([128, 128], BF16, tag="pt", bufs=2,
                                        padded_shape=[128, 1024])
                        nc.tensor.transpose(pst[:], h1t[:, q * 128:(q + 1) * 128],
                                            ident_sb[:])
                        nc.vector.tensor_copy(
                            h1Ts[h * 4 + q][:, bt * 128:(bt + 1) * 128], pst[:])
                        psb2 = psum.tile([128, 128], BF16, tag="pt", bufs=2,
                                         padded_shape=[128, 1024])
                        nc.tensor.transpose(psb2[:], h1b[:, q * 128:(q + 1) * 128],
                                            ident_sb[:])
                        nc.vector.tensor_copy(
                            h1Ts[8 + h * 4 + q][:, bt * 128:(bt + 1) * 128],
                            psb2[:])

            # ---------- head ----------
            if not do_pe:
                zz = const.tile([1, BC], F32)
                nc.vector.memset(zz[:], 0.0)
                nc.sync.dma_start(out.ap()[0:1, :], zz[:])
                return
            ps1 = psum.tile([L1, BC], F32, tag="ps")
            for t in range(16):
                nc.tensor.matmul(ps1[:], l1w_sb[:, t, :], h1Ts[t][:],
                                 start=(t == 0), stop=(t == 15))
            h2f = tmppool.tile([L1, BC], F32, tag="h2f")
            nc.vector.tensor_scalar(h2f[:], ps1[:], l1b_sb[:], 0.0,
                                    Alu.add, Alu.max)
            h2 = tmppool.tile([L1, BC], BF16, tag="h2")
            nc.vector.tensor_scalar(h2[:], h2f[:], 1.0, None, Alu.min)

            ps2 = psum.tile([L2, BC], F32, tag="ps")
            nc.tensor.matmul(ps2[:], l2w_sb[:], h2[:], start=True, stop=True)
            h3f = tmppool.tile([L2, BC], F32, tag="h3f")
            nc.vector.tensor_scalar(h3f[:], ps2[:], l2b_sb[:], 0.0,
                                    Alu.add, Alu.max)
            h3 = tmppool.tile([L2, BC], BF16, tag="h3")
            nc.vector.tensor_scalar(h3[:], h3f[:], 1.0, None, Alu.min)

            ps3 = psum.tile([1, BC], F32, tag="ps")
            nc.tensor.matmul(ps3[:], l3w_sb[:], h3[:], start=True, stop=True)

            sig_sb = const.tile([1, BC], F32)
            raw_sb = const.tile([1, BC], F32)
            nc.vector.tensor_scalar(raw_sb[:], ps3[:], l3b_sb[:], None,
                                    Alu.add)
            nc.scalar.activation(sig_sb[:], ps3[:], Act.Sigmoid,
                                 bias=l3b_sb[:])
            nc.sync.dma_start(out.ap()[0:1, :], sig_sb[:])
            nc.sync.dma_start(out.ap()[1:2, :], raw_sb[:])

        for _rep in range(reps):
            emit_body()

    nc.compile()
    return nc


def _row_indices(feat_mat):
    """Per-row sorted active-feature indices of a 0/1 matrix."""
    rows, cols = np.nonzero(feat_mat)
    counts = np.bincount(rows, minlength=feat_mat.shape[0])
    split = np.cumsum(counts)[:-1]
    return np.split(cols, split), counts


def _wrap_idx(a):
    """Slot i -> [i % 16, i // 16], replicated to 128 partitions (int16)."""
    m = np.ascontiguousarray(a.reshape(-1, 16).T.astype(np.int16))
    return np.tile(m, (8, 1))


def _wrap_seg(a):
    """Slot i -> [i % 128, i // 128] (f32)."""
    return np.ascontiguousarray(a.reshape(-1, 128).T.astype(np.float32))


def _prep_sparse(white_features, black_features, stm, ft_w, ft_b,
                 l1_w, l1_b, l2_w, l2_b, l3_w, l3_b):
    """Host prep: bin-pack rows, build tables + index streams.

    Returns (ga, gb, in_maps, perm) or None if the inputs don't fit the
    sparse path.
    """
    import heapq
    f32 = lambda a: np.ascontiguousarray(np.asarray(a, dtype=np.float32))

    white = np.asarray(white_features)
    black = np.asarray(black_features)
    widx, wcnt = _row_indices(white)
    bidx, bcnt = _row_indices(black)

    # bin-pack rows into 32 (core, bt) bins of 128 rows, balancing totals
    tot = wcnt + bcnt
    nbins = NCORES * BT
    order = np.argsort(-tot, kind="stable")
    heap = [(0, b) for b in range(nbins)]
    heapq.heapify(heap)
    bins = [[] for _ in range(nbins)]
    spill = []
    for r in order:
        load, b = heapq.heappop(heap)
        bins[b].append(int(r))
        if len(bins[b]) < 128:
            heapq.heappush(heap, (load + int(tot[r]), b))
        else:
            spill.append((load + int(tot[r]), b))
    assert all(len(rows) == 128 for rows in bins)

    # build streams per (core, bt, persp, half)
    streams_lo, streams_hi = {}, {}
    max_lo = max_hi = 0
    for b in range(nbins):
        rows = bins[b]
        for p, idx_lists in enumerate((widx, bidx)):
            lo_i, lo_s, hi_i, hi_s = [], [], [], []
            for j, r in enumerate(rows):
                ii = idx_lists[r]
                lo = ii[ii < SPLIT]
                hi = ii[ii >= SPLIT] - SPLIT
                lo_i.append(lo)
                lo_s.append(np.full(len(lo), j))
                hi_i.append(hi)
                hi_s.append(np.full(len(hi), j))
            lo_i = np.concatenate(lo_i) if lo_i else np.empty(0, np.int64)
            lo_s = np.concatenate(lo_s) if lo_s else np.empty(0, np.int64)
            hi_i = np.concatenate(hi_i) if hi_i else np.empty(0, np.int64)
            hi_s = np.concatenate(hi_s) if hi_s else np.empty(0, np.int64)
            streams_lo[(b, p)] = (lo_i, lo_s)
            streams_hi[(b, p)] = (hi_i, hi_s)
            max_lo = max(max_lo, len(lo_i))
            max_hi = max(max_hi, len(hi_i))

    ga = max(1, math.ceil(max_lo / 128))
    gb = math.ceil(max_hi / 128)
    if ga + gb > GCAP:
        return None

    # pad streams and pack into per-core column-block arrays
    def pack(streams, g, zrow):
        cols_i, cols_s = [], []
        for b in range(nbins):
            for p in range(2):
                ii, ss = streams[(b, p)]
                n = g * 128
                pi = np.full(n, zrow, np.int64)
                pi[:len(ii)] = ii
                psg = np.zeros(n, np.int64)
                psg[:len(ss)] = ss
                cols_i.append(_wrap_idx(pi))
                cols_s.append(_wrap_seg(psg))
        # group per core: bins are (core*BT + bt)
        per_core_i, per_core_s = [], []
        for c in range(NCORES):
            blocks = [c * BT * 2 + k for k in range(BT * 2)]
            per_core_i.append(np.concatenate([cols_i[k] for k in blocks],
                                             axis=1))
            per_core_s.append(np.concatenate([cols_s[k] for k in blocks],
                                             axis=1))
        return per_core_i, per_core_s

    idxlo_c, seglo_c = pack(streams_lo, ga, ZLO)
    if gb:
        idxhi_c, seghi_c = pack(streams_hi, gb, ZHI)

    # tables (bf16)
    wT = np.asarray(ft_w, dtype=np.float32).T  # [FEAT, HID]
    tbl = np.zeros((NTA + NTB, HID), BF16_NP)
    tbl[0:SPLIT] = wT[0:SPLIT].astype(BF16_NP)
    tbl[NTA:NTA + (FEAT - SPLIT)] = wT[SPLIT:FEAT].astype(BF16_NP)

    biasb = np.tile(np.asarray(ft_b, np.float32).astype(BF16_NP), (128, 1))
    ident = np.eye(128, dtype=BF16_NP)
    iota = np.tile(np.arange(128, dtype=np.float32), (128, 1))

    stm_flat = np.asarray(stm, dtype=np.float32).reshape(B)
    bfc = lambda a: np.ascontiguousarray(
        np.asarray(a, dtype=np.float32).astype(BF16_NP))
    l1wT = bfc(np.asarray(l1_w, dtype=np.float32).T)
    l2wT = bfc(np.asarray(l2_w, dtype=np.float32).T)
    l3wT = bfc(np.asarray(l3_w, dtype=np.float32).T)

    perm = np.zeros((NCORES, BC), np.int64)
    in_maps = []
    for c in range(NCORES):
        rows_c = np.concatenate([bins[c * BT + bt] for bt in range(BT)])
        perm[c] = rows_c
        stm_c = np.ascontiguousarray(
            stm_flat[rows_c].reshape(BT, 128).T.astype(np.float32))
        m = dict(
            tbl=tbl, idxlo=idxlo_c[c], seglo=seglo_c[c],
            stm4=stm_c, iota=iota, ident=ident, biasb=biasb,
            l1w=l1wT, l1b=f32(l1_b), l2w=l2wT, l2b=f32(l2_b),
            l3w=l3wT, l3b=f32(l3_b))
        if gb:
            m["idxhi"] = idxhi_c[c]
            m["seghi"] = seghi_c[c]
        in_maps.append(m)
    return ga, gb, in_maps, perm


def _is_binary(x):
    x = np.asarray(x)
    s = x.ravel()[:: max(1, x.size // 65536)]
    if not np.all((s == 0) | (s == 1)):
        return False
    return bool(np.all((x == 0) | (x == 1)))


# --------------------------------------------------------------------------
# Dense fallback path (original kernel)
# --------------------------------------------------------------------------

def _build(feat=FEAT, gk=16, mode="full", reps=1):
    """Build + compile the dense per-core Bass program. Returns nc."""
    kt = feat // 128          # number of K tiles
    assert kt % gk == 0
    ng = kt // gk             # number of K groups
    Alu = mybir.AluOpType
    Act = mybir.ActivationFunctionType

    nc = bacc.Bacc("TRN2", target_bir_lowering=False, debug=False,
                   num_devices=NCORES)

    fw = nc.dram_tensor("fw", (feat, BC), F32, kind="ExternalInput")
    fb = nc.dram_tensor("fb", (feat, BC), F32, kind="ExternalInput")
    wT = nc.dram_tensor("wT", (feat, HID), F32, kind="ExternalInput")
    ftb = nc.dram_tensor("ftb", (HID,), F32, kind="ExternalInput")
    stm = nc.dram_tensor("stm", (BC,), F32, kind="ExternalInput")
    l1w = nc.dram_tensor("l1w", (2 * HID, L1), F32, kind="ExternalInput")
    l1b = nc.dram_tensor("l1b", (L1,), F32, kind="ExternalInput")
    l2w = nc.dram_tensor("l2w", (L1, L2), F32, kind="ExternalInput")
    l2b = nc.dram_tensor("l2b", (L2,), F32, kind="ExternalInput")
    l3w = nc.dram_tensor("l3w", (L2, 1), F32, kind="ExternalInput")
    l3b = nc.dram_tensor("l3b", (1,), F32, kind="ExternalInput")
    out = nc.dram_tensor("out", (2, BC), F32, kind="ExternalOutput")

    with ExitStack() as ctx:
        tc = ctx.enter_context(tile.TileContext(nc))
        const = ctx.enter_context(tc.tile_pool(name="const", bufs=1))
        wpool = ctx.enter_context(tc.tile_pool(name="wpool", bufs=2))
        fwpool = ctx.enter_context(tc.tile_pool(name="fwpool", bufs=2))
        fbpool = ctx.enter_context(tc.tile_pool(name="fbpool", bufs=2))
        accpool = ctx.enter_context(tc.tile_pool(name="accpool", bufs=1))
        h1pool = ctx.enter_context(tc.tile_pool(name="h1pool", bufs=1))
        tmppool = ctx.enter_context(tc.tile_pool(name="tmppool", bufs=1))
        psum = ctx.enter_context(
            tc.tile_pool(name="psum", bufs=8, space="PSUM"))

        # ---------- constants ----------
        ftb_sb = const.tile([128, NHC], F32)
        nc.sync.dma_start(ftb_sb[:], ftb.ap().rearrange("(c p) -> p c", p=128))
        l1b_sb = const.tile([L1, 1], F32)
        nc.sync.dma_start(l1b_sb[:], l1b.ap())
        l2b_sb = const.tile([L2, 1], F32)
        nc.sync.dma_start(l2b_sb[:], l2b.ap())
        l3b_sb = const.tile([1, 1], F32)
        nc.sync.dma_start(l3b_sb[:], l3b.ap())

        l1w_sb = const.tile([128, (2 * HID) // 128, L1], BF16)
        nc.gpsimd.dma_start(l1w_sb[:],
                            l1w.ap().rearrange("(t p) m -> p t m", p=128))
        l2w_sb = const.tile([L1, L2], BF16)
        nc.gpsimd.dma_start(l2w_sb[:], l2w.ap())
        l3w_sb = const.tile([L2, 1], BF16)
        nc.gpsimd.dma_start(l3w_sb[:], l3w.ap())

        stm_bf = const.tile([1, BC], BF16)
        nc.gpsimd.dma_start(stm_bf[:], stm.ap())
        ones_bf = const.tile([1, 128], BF16)
        nc.vector.memset(ones_bf[:], 1.0)

        # broadcast stm across partitions: [128, BC] = ones[1,128].T @ stm[1,BC]
        ps_stm = psum.tile([128, BC], F32, tag="ps")
        nc.tensor.matmul(ps_stm[:], ones_bf[:], stm_bf[:],
                         start=True, stop=True)
        stmb_sb = const.tile([128, BC], F32)
        nc.vector.tensor_copy(stmb_sb[:], ps_stm[:])

        # persistent fp32 accumulators: [0..7] = white persp, [8..15] = black
        accs = [accpool.tile([128, BC], F32, tag=f"acc{i}", name=f"acc{i}")
                for i in range(16)]

        # ---------- feature transformer main loop ----------
        def emit_body():
            sched = [gk] * ng
            roff = 0
            for g, gsz in enumerate(sched):
                r0, r1 = roff * 128, (roff + gsz) * 128
                roff += gsz
                wt = wpool.tile([128, gsz, HID], BF16, tag="wt",
                                name="wt")
                nc.gpsimd.dma_start(
                    wt[:],
                    wT.ap()[r0:r1, :].rearrange("(t p) h -> p t h", p=128))
                fwt = fwpool.tile([128, gsz, BC], BF16, tag="fwt",
                                  name="fwt")
                nc.gpsimd.dma_start(
                    fwt[:],
                    fw.ap()[r0:r1, :].rearrange("(t p) n -> p t n", p=128))
                fbt = fbpool.tile([128, gsz, BC], BF16, tag="fbt",
                                  name="fbt")
                nc.gpsimd.dma_start(
                    fbt[:],
                    fb.ap()[r0:r1, :].rearrange("(t p) n -> p t n", p=128))

                for s, ftile in enumerate((fwt, fbt)):
                    for c in range(NHC):
                        ps = psum.tile([128, BC], F32, tag="ps")
                        for t in range(gsz):
                            nc.tensor.matmul(
                                ps[:],
                                wt[:, t, c * 128:(c + 1) * 128],
                                ftile[:, t, :],
                                start=(t == 0), stop=(t == gsz - 1))
                        a = accs[s * NHC + c]
                        if g == 0:
                            nc.vector.tensor_scalar(
                                a[:], ps[:], ftb_sb[:, c:c + 1], None,
                                Alu.add)
                        else:
                            nc.vector.tensor_add(a[:], a[:], ps[:])

            # ---------- stm select + clip -> h1 (bf16) ----------
            h1s = [h1pool.tile([128, BC], BF16, tag=f"h1_{i}", name=f"h1_{i}")
                   for i in range(16)]
            for c in range(NHC):
                w_, b_ = accs[c], accs[NHC + c]
                d = tmppool.tile([128, BC], F32, tag="d")
                nc.vector.tensor_sub(d[:], w_[:], b_[:])
                m = tmppool.tile([128, BC], F32, tag="m")
                nc.vector.tensor_mul(m[:], d[:], stmb_sb[:])
                topf = tmppool.tile([128, BC], F32, tag="topf")
                nc.vector.tensor_add(topf[:], b_[:], m[:])
                botf = tmppool.tile([128, BC], F32, tag="botf")
                nc.vector.tensor_sub(botf[:], w_[:], m[:])
                nc.gpsimd.tensor_scalar(
                    h1s[c][:], topf[:], 0.0, 1.0, Alu.max, Alu.min)
                nc.gpsimd.tensor_scalar(
                    h1s[NHC + c][:], botf[:], 0.0, 1.0, Alu.max, Alu.min)

            # ---------- head ----------
            ps1 = psum.tile([L1, BC], F32, tag="ps")
            for t in range(16):
                nc.tensor.matmul(ps1[:], l1w_sb[:, t, :], h1s[t][:],
                                 start=(t == 0), stop=(t == 15))
            h2f = tmppool.tile([L1, BC], F32, tag="h2f")
            nc.vector.tensor_scalar(h2f[:], ps1[:], l1b_sb[:], 0.0, Alu.add, Alu.max)
            h2 = tmppool.tile([L1, BC], BF16, tag="h2")
            nc.vector.tensor_scalar(h2[:], h2f[:], 1.0, None, Alu.min)

            ps2 = psum.tile([L2, BC], F32, tag="ps")
            nc.tensor.matmul(ps2[:], l2w_sb[:], h2[:], start=True, stop=True)
            h3f = tmppool.tile([L2, BC], F32, tag="h3f")
            nc.vector.tensor_scalar(h3f[:], ps2[:], l2b_sb[:], 0.0, Alu.add, Alu.max)
            h3 = tmppool.tile([L2, BC], BF16, tag="h3")
            nc.vector.tensor_scalar(h3[:], h3f[:], 1.0, None, Alu.min)

            ps3 = psum.tile([1, BC], F32, tag="ps")
            nc.tensor.matmul(ps3[:], l3w_sb[:], h3[:], start=True, stop=True)

            sig_sb = const.tile([1, BC], F32)
            raw_sb = const.tile([1, BC], F32)
            nc.vector.tensor_scalar(raw_sb[:], ps3[:], l3b_sb[:], None, Alu.add)
            nc.scalar.activation(sig_sb[:], ps3[:], Act.Sigmoid, bias=l3b_sb[:])
            nc.sync.dma_start(out.ap()[0:1, :], sig_sb[:])
            nc.sync.dma_start(out.ap()[1:2, :], raw_sb[:])

        for _rep in range(reps):
            emit_body()

    nc.compile()
    return nc


def _prep_in_maps(white_features, black_features, stm, ft_w, ft_b,
                  l1_w, l1_b, l2_w, l2_b, l3_w, l3_b):
    f32 = lambda a: np.ascontiguousarray(np.asarray(a, dtype=np.float32))
    white = np.asarray(white_features, dtype=np.float32)
    black = np.asarray(black_features, dtype=np.float32)
    stm = np.asarray(stm, dtype=np.float32).reshape(B)
    wT = f32(np.asarray(ft_w, dtype=np.float32).T)        # [FEAT, HID]
    l1wT = f32(np.asarray(l1_w, dtype=np.float32).T)      # [2048, 64]
    l2wT = f32(np.asarray(l2_w, dtype=np.float32).T)      # [64, 32]
    l3wT = f32(np.asarray(l3_w, dtype=np.float32).T)      # [32, 1]
    ftb = f32(ft_b)
    l1b, l2b, l3b = f32(l1_b), f32(l2_b), f32(l3_b)

    in_maps = []
    for c in range(NCORES):
        sl = slice(c * BC, (c + 1) * BC)
        in_maps.append(dict(
            fw=f32(white[sl].T), fb=f32(black[sl].T), wT=wT, ftb=ftb,
            stm=f32(stm[sl]), l1w=l1wT, l1b=l1b, l2w=l2wT, l2b=l2b,
            l3w=l3wT, l3b=l3b))
    return in_maps


# --------------------------------------------------------------------------
# entry point
# --------------------------------------------------------------------------

def kernel(**inputs):
    white = np.asarray(inputs["white_features"])
    black = np.asarray(inputs["black_features"])
    use_sparse = (white.shape == (B, FEAT) and black.shape == (B, FEAT)
                  and _is_binary(white) and _is_binary(black))
    prep = None
    if use_sparse:
        prep = _prep_sparse(**inputs)
    if prep is not None:
        ga, gb, in_maps, perm = prep
        key = ("sparse", ga, gb)
        if key not in _CACHE:
            _CACHE[key] = _build_sparse(ga, gb)
        _CACHE["last_sparse"] = (ga, gb, in_maps)
        nc = _CACHE[key]
        res = run_bass_kernel_spmd(nc, in_maps, core_ids=list(range(NCORES)))
        sig = np.zeros(B, np.float32)
        raw = np.zeros(B, np.float32)
        for c in range(NCORES):
            sig[perm[c]] = res.results[c]["out"][0]
            raw[perm[c]] = res.results[c]["out"][1]
        return (sig.reshape(B, 1), raw.reshape(B, 1))

    # dense fallback
    if "dense" not in _CACHE:
        _CACHE["dense"] = _build()
    nc = _CACHE["dense"]
    in_maps = _prep_in_maps(**inputs)
    res = run_bass_kernel_spmd(nc, in_maps, core_ids=list(range(NCORES)))
    sig = np.concatenate([res.results[c]["out"][0] for c in range(NCORES)])
    raw = np.concatenate([res.results[c]["out"][1] for c in range(NCORES)])
    return (sig.reshape(B, 1).astype(np.float32),
            raw.reshape(B, 1).astype(np.float32))
